# revision 49
# baseline (speedup 1.0000x reference)
"""BitNet-style MLP (rmsnorm -> act-quant -> ternary fc1 -> StarReLU ->
rmsnorm -> act-quant -> ternary fc2) on 8 Trainium2 NeuronCores.

Sharding: data-parallel over tokens (8192 tokens -> 1024/core). Weights are
replicated; the global mean(|w|) statistic is computed cooperatively (each
core reduces 1/8 of each weight, then a tiny AllReduce), after which every
core quantizes the full weight set to ternary fp8 on the fly.

Numeric scheme (identical to the reference up to fp32 rounding of scales):
the fake-quantized matmuls are integer-exact, so both matmuls run on the PE
in bf16 (activations = round(x*scale), ints in [-128,127], exact in bf16)
x fp8 (ternary weights in {-1,0,1}), with exact f32 accumulation in PSUM;
per-row dequant scales are applied at PSUM eviction. Rounding uses the
(x + 1.5*2^23) - 1.5*2^23 magic-number trick (RNE, matches jnp.round).
Weight clip uses clip-BEFORE-round at +-1.49 which is exactly equivalent to
clip(round(v),-1,1) under RNE.

Engine assignment notes (perf):
 - NO elementwise work on GpSimd: GpSimd ops and DVE 2-port perf-mode ops
   take an exclusive lock on a shared SBUF port pair and fully serialize
   against each other. Everything elementwise runs on DVE or ACT (ACT has
   its own SBUF ports).
 - DMA transposes are batched: ONE dma_start_transpose per [128, N] tile
   with a 3D [128, N/128, 128] destination (instead of N/128 separate
   128x128 calls).
 - The tiny AllReduce is issued early and its (DVE) post-processing is
   emitted after the first X tiles so the in-order DVE queue doesn't stall
   on the collective.
"""

import numpy as np
from contextlib import ExitStack

import concourse.bass as bass
import concourse.tile as tile
from concourse import bacc, mybir
from concourse.bass_utils import run_bass_kernel_spmd

AO = mybir.AluOpType
AF = mybir.ActivationFunctionType
F32 = mybir.dt.float32
BF16 = mybir.dt.bfloat16
FP8 = mybir.dt.float8e4

MAGIC = 1.5 * 2 ** 23  # RNE integer rounding for |v| <= 2^22
CLIP = 1.49            # clip-before-round bound; == clip(round(v),-1,1) after RNE
N_CORES = 8
B, S, D, H = 4, 2048, 1024, 4096
TOK_TOTAL = B * S                 # 8192
TOK = TOK_TOTAL // N_CORES        # 1024 tokens per core
NT = TOK // 128                   # 8 token tiles per core
DK = D // 128                     # 8 k-tiles for fc1
HK = H // 128                     # 32 k-tiles for fc2
HN = H // 512                     # 8 n-chunks for fc1
DN = D // 512                     # 2 n-chunks for fc2
EPS_NORM = 1e-08
EPS_Q = 1e-05

DEFAULT_CFG = dict(
    big_bufs=2,
    xmid_bufs=2,
    y_bufs=2,
    stats_bufs=6,
    ws1_bufs=2,   # shared "wst" tag ring: slice tiles + w1 half-row pieces
    ws2_bufs=2,
    hq_bufs=2,
    # 8 xq bufs: X(2..7) transposes serialize against the collective (Tile
    # guards DMA-transpose vs collective), so xq tiles pile up until the
    # AllReduce lands; the X ACT/DVE pipeline must not block on the ring.
    xq_bufs=8,
    xt_bufs=3,
    ht_bufs=2,
    fc1_ps_bufs=5,
    fc2_ps_bufs=2,
)


def _step0_free_ap(t_ap: bass.AP, count: int) -> bass.AP:
    """AP that 'writes' `count` elements per partition all landing on the
    same address (free step 0) - a bit bucket for ops whose only useful
    output is accum_out."""
    part = t_ap.ap[0]
    return bass.AP(t_ap.tensor, t_ap.offset, [[part[0], part[1]], [0, count]])


def build(s_act: float, b_act: float, use_n1: bool, use_n2: bool, cfg=None):
    """Build the per-core Bass module (SPMD: same module on all 8 cores)."""
    cfg = dict(DEFAULT_CFG, **(cfg or {}))
    general = use_n1 or use_n2 or (b_act != 0.0) or (s_act < 0.0)
    fast_act = (b_act == 0.0) and (s_act >= 0.0)

    nc = bacc.Bacc(None, target_bir_lowering=False)

    x_d = nc.dram_tensor("x", [TOK, D], F32, kind="ExternalInput")
    w1t_d = nc.dram_tensor("w1t", [D, H], F32, kind="ExternalInput")
    w2t_d = nc.dram_tensor("w2t", [H, D], F32, kind="ExternalInput")
    w1s_d = nc.dram_tensor("w1s", [D // N_CORES, H], F32, kind="ExternalInput")
    w2s_d = nc.dram_tensor("w2s", [H // N_CORES, D], F32, kind="ExternalInput")
    if use_n1:
        n1_d = nc.dram_tensor("n1", [1, D], F32, kind="ExternalInput")
    if use_n2:
        n2_d = nc.dram_tensor("n2", [1, H], F32, kind="ExternalInput")
    y_d = nc.dram_tensor("y", [TOK, D], F32, kind="ExternalOutput")

    with tile.TileContext(nc) as tc:
        with ExitStack() as ctx:
            const = ctx.enter_context(tc.tile_pool(name="const", bufs=1))
            stats = ctx.enter_context(tc.tile_pool(name="stats", bufs=cfg["stats_bufs"]))
            wq_pool = ctx.enter_context(tc.tile_pool(name="wq", bufs=1))
            ws1 = ctx.enter_context(tc.tile_pool(name="ws1", bufs=cfg["ws1_bufs"]))
            ws2 = ctx.enter_context(tc.tile_pool(name="ws2", bufs=cfg["ws2_bufs"]))
            big = ctx.enter_context(tc.tile_pool(name="big", bufs=cfg["big_bufs"]))
            xmid = ctx.enter_context(tc.tile_pool(name="xmid", bufs=cfg["xmid_bufs"]))
            ymid = ctx.enter_context(tc.tile_pool(name="ymid", bufs=cfg["y_bufs"]))
            xq_pool = ctx.enter_context(tc.tile_pool(name="xq", bufs=cfg["xq_bufs"]))
            xt_pool = ctx.enter_context(tc.tile_pool(name="xT", bufs=cfg["xt_bufs"]))
            hq_pool = ctx.enter_context(tc.tile_pool(name="hq", bufs=cfg["hq_bufs"]))
            ht_pool = ctx.enter_context(tc.tile_pool(name="hT", bufs=cfg["ht_bufs"]))
            junk_pool = ctx.enter_context(tc.tile_pool(name="junk", bufs=1))
            fc1_ps = ctx.enter_context(tc.tile_pool(name="fc1ps", bufs=cfg["fc1_ps_bufs"], space="PSUM"))
            fc2_ps = ctx.enter_context(tc.tile_pool(name="fc2ps", bufs=cfg["fc2_ps_bufs"], space="PSUM"))
            ps_small = ctx.enter_context(tc.tile_pool(name="pssmall", bufs=1, space="PSUM"))
            dram = ctx.enter_context(tc.tile_pool(name="dram", bufs=1, space="DRAM"))

            ones_col = const.tile([128, 1], F32)
            nc.vector.memset(ones_col[:], 1.0)
            ones_row = const.tile([1, 128], F32)
            nc.vector.memset(ones_row[:], 1.0)
            magic_b = const.tile([128, 1], F32)
            nc.vector.memset(magic_b[:], MAGIC)
            nmagic_b = const.tile([128, 1], F32)
            nc.vector.memset(nmagic_b[:], -MAGIC)

            # wstats columns: 0=mw1, 1=mw2, 2=s1, 3=s2
            wstats = stats.tile([128, 4], F32, tag="wstats")
            mw1_ap, mw2_ap = wstats[:, 0:1], wstats[:, 1:2]
            s1_ap, s2_ap = wstats[:, 2:3], wstats[:, 3:4]

            junk_small = junk_pool.tile([128, 8], BF16, tag="junk")

            def junk_ap(n):
                return _step0_free_ap(junk_small[:, 0:1], n)

            if use_n1:
                n1_sb = const.tile([128, D], F32)
                a0 = n1_d[:]
                nc.sync.dma_start(n1_sb[:], bass.AP(a0.tensor, a0.offset, [[0, 128], [1, D]]))
            if use_n2:
                n2_sb = const.tile([128, H], F32)
                a0 = n2_d[:]
                nc.sync.dma_start(n2_sb[:], bass.AP(a0.tensor, a0.offset, [[0, 128], [1, H]]))

            # -------- W phase: global mean(|w|) via ONE tiny AllReduce --------
            # In steady state the collective doubles as a barrier that
            # re-aligns the 8 cores each iteration, so its cost is the true
            # AR latency (~20-40us), not the first-exec launch skew.
            ar_in = dram.tile([4, 1], F32, tag="ari")
            ar_out = dram.tile([4, 1], F32, tag="aro")

            def weight_means_reduce():
                # 4 pipelined load+reduce pieces (w1 halves -> cols 0/2,
                # w2 halves -> cols 1/3) so the AllReduce input is ready
                # ~15us in instead of ~30us.
                # 4 pipelined load+reduce pieces (w1 halves -> cols 0/2,
                # w2 halves -> cols 1/3) so the AllReduce input is ready
                # ~15us in instead of ~30us.
                parts = stats.tile([128, 4], F32, tag="wpart")
                pieces = []
                halfc = w1s_d.shape[1] // 2      # w1s split by columns (128 rows)
                half2 = w2s_d.shape[0] // 2      # w2s split by rows (512 rows)
                pieces.append((0, w1s_d[:, 0:halfc]))
                pieces.append((1, w2s_d[0:half2, :]))
                pieces.append((2, w1s_d[:, halfc:]))
                pieces.append((3, w2s_d[half2:, :]))
                for col, sl in pieces:
                    st_t = ws1.tile([128, sl.shape[0] // 128, sl.shape[1]],
                                    F32, tag="wst", name=f"wst{col}")
                    nc.sync.dma_start(st_t[:], sl.rearrange("(c p) f -> p c f", p=128))
                    nc.vector.tensor_reduce(parts[:, col:col + 1], st_t[:],
                                            mybir.AxisListType.XY, AO.add,
                                            apply_absolute_value=True)
                # cross-partition sums: [4,1] = parts.T @ ones
                pss = ps_small.tile([128, 1], F32, tag="ps", name="pssum")
                nc.tensor.matmul(pss[0:4, :], parts[:], ones_col[:], start=True, stop=True)
                sb1 = stats.tile([4, 1], F32, tag="wsum")
                nc.scalar.activation(sb1[:], pss[0:4, :], AF.Copy)
                nc.sync.dma_start(ar_in[:], sb1[:])

            def weight_means_collective():
                nc.gpsimd.collective_compute(
                    "AllReduce", AO.add,
                    replica_groups=[list(range(N_CORES))],
                    ins=[ar_in[:].opt()], outs=[ar_out[:].opt()],
                )

            def weight_means_post():
                bc = stats.tile([128, 4], F32, tag="wbc")
                a0 = ar_out[:]
                # issued on the ACT HWDGE ring: the SP ring is jammed with
                # DMA-transposes that serialize against the collective.
                nc.scalar.dma_start(bc[:], bass.AP(a0.tensor, a0.offset, [[0, 128], [1, 4]]))
                bsum = stats.tile([128, 2], F32, tag="wbsum")
                nc.vector.tensor_tensor(bsum[:], bc[:, 0:2], bc[:, 2:4], AO.add)
                nc.vector.tensor_scalar(wstats[:, 0:2], bsum[:], 1.0 / (D * H), EPS_Q,
                                        AO.mult, AO.max)
                nc.vector.reciprocal(wstats[:, 2:4], wstats[:, 0:2])

            # ---------------- weight quantization (full, streamed) -----------
            # 3 passes, zero GpSimd:
            #   DVE: v = min(w*s, 1.49)        (2-op tensor_scalar, in-place)
            #   DVE: v = max(v, -1.49) + MAGIC (2-op tensor_scalar, in-place)
            #   ACT: wq = v - MAGIC -> fp8     (Identity with bias=-MAGIC)
            # w1 is processed in [128, H/2] half-row pieces so FC1(0)'s k-loop
            # can start consuming quantized chunks while later ones stream.
            # Loads are decoupled from quants so DMAs prefetch `bufs` deep.
            w1q = wq_pool.tile([128, DK, H], FP8, tag="w1q")
            w2q = wq_pool.tile([128, HK, D], FP8, tag="w2q")
            HH2 = H // 2
            w1_tiles = {}
            w2_tiles = {}

            def emit_w1_load(k, j):
                wt = ws1.tile([128, HH2], F32, tag="wst")
                nc.sync.dma_start(wt[:], w1t_d[128 * k:128 * (k + 1),
                                               HH2 * j:HH2 * (j + 1)])
                w1_tiles[(k, j)] = wt

            def emit_w1_quant(k, j):
                # quantize in 512-col quarters (the exact region one FC1
                # matmul reads): each matmul unblocks on its own quarter,
                # keeping PE stalls under the 3.4us HAM re-throttle window
                # while the weight stream paces the pipeline.
                wt = w1_tiles.pop((k, j))
                for q in range(HH2 // 512):
                    sl = slice(512 * q, 512 * (q + 1))
                    nc.vector.tensor_scalar(wt[:, sl], wt[:, sl], s1_ap, CLIP,
                                            AO.mult, AO.min)
                    nc.vector.tensor_scalar(wt[:, sl], wt[:, sl], -CLIP, MAGIC,
                                            AO.max, AO.add)
                    nc.scalar.activation(
                        w1q[:, k, HH2 * j + 512 * q:HH2 * j + 512 * (q + 1)],
                        wt[:, sl], AF.Identity, bias=nmagic_b[:])

            def emit_w2_load(k):
                wt = ws2.tile([128, D], F32, tag="ws2")
                nc.sync.dma_start(wt[:], w2t_d[128 * k:128 * (k + 1), :])
                w2_tiles[k] = wt

            def emit_w2_quant(k):
                # 512-col halves for the same reason as emit_w1_quant.
                wt = w2_tiles.pop(k)
                for q in range(D // 512):
                    sl = slice(512 * q, 512 * (q + 1))
                    nc.vector.tensor_scalar(wt[:, sl], wt[:, sl], s2_ap, CLIP,
                                            AO.mult, AO.min)
                    nc.vector.tensor_scalar(wt[:, sl], wt[:, sl], -CLIP, MAGIC,
                                            AO.max, AO.add)
                    nc.scalar.activation(w2q[:, k, sl], wt[:, sl],
                                         AF.Identity, bias=nmagic_b[:])

            # ---------------- main token-tile pipeline ------------------------
            tiles = [dict() for _ in range(NT)]

            def rsqrt_newton(st, u, q, rs0):
                """rs = 1/sqrt(u), Newton-refined to ~1ulp (ACT Sqrt alone is
                only ~8ulp and rounding-boundary flips are costly here)."""
                nA, nB, rsr = st[:, 12:13], st[:, 13:14], st[:, 14:15]
                nc.scalar.activation(q, u, AF.Sqrt)
                nc.vector.reciprocal(rs0, q)
                nc.vector.tensor_tensor(nA, rs0, rs0, AO.mult)
                nc.vector.tensor_tensor(nB, nA, u, AO.mult)
                nc.vector.tensor_scalar(nA, nB, -0.5, 1.5, AO.mult, AO.add)
                nc.vector.tensor_tensor(rsr, rs0, nA, AO.mult)
                return rsr

            def emit_d1(i):
                # d1 = amax * mw1 * k; depends on the AllReduce (mw1), so for
                # tiles 0/1 it is emitted from weight_means_post, for later
                # tiles at the end of emit_X.
                t = tiles[i]
                st = t["st"]
                amax, d1 = st[:, 5:6], st[:, 8:9]
                d1_k = (np.sqrt(s_act) if fast_act else 1.0) / 127.0
                nc.vector.tensor_scalar(d1, amax, mw1_ap, d1_k, AO.mult, AO.mult)

            def emit_X(i):
                t = tiles[i]
                xt = xmid.tile([128, D], F32, tag="x")
                nc.sync.dma_start(xt[:], x_d[128 * i:128 * (i + 1), :])
                st = stats.tile([128, 32], F32, tag="st", bufs=NT)
                ssum, m1 = st[:, 0:1], st[:, 1:2]
                u, q, rs = st[:, 2:3], st[:, 3:4], st[:, 4:5]
                amax, ia = st[:, 5:6], st[:, 6:7]
                c1, d1 = st[:, 7:8], st[:, 8:9]
                nc.scalar.activation(junk_ap(D), xt[:], AF.Square, accum_out=ssum)
                if use_n1:
                    xw = xmid.tile([128, D], F32, tag="xw")
                    nc.vector.tensor_tensor(xw[:], xt[:], n1_sb[:], AO.mult)
                    quant_src = xw
                else:
                    quant_src = xt
                nc.vector.tensor_reduce(m1, quant_src[:], mybir.AxisListType.X,
                                        AO.max, apply_absolute_value=True)
                nc.vector.tensor_scalar(u, ssum, 1.0 / D, EPS_NORM, AO.mult, AO.add)
                rs = rsqrt_newton(st, u, q, rs)
                nc.vector.tensor_scalar(amax, m1, rs, EPS_Q, AO.mult, AO.max)
                nc.vector.reciprocal(ia, amax)
                nc.vector.tensor_scalar(c1, ia, rs, 127.0, AO.mult, AO.mult)
                nc.vector.tensor_scalar(quant_src[:], quant_src[:], c1, MAGIC, AO.mult, AO.add)
                xq = xq_pool.tile([128, D], BF16, tag="xq")
                nc.scalar.activation(xq[:], quant_src[:], AF.Identity, bias=nmagic_b[:])
                t["xq"], t["st"], t["d1"] = xq, st, d1

            def emit_XT(i):
                # transposes serialize against the collective (Tile guard):
                # without a timing hint the scheduler slots them EARLY on the
                # SP ring (its cost model assumes the collective is instant),
                # blocking the x loads behind the AllReduce. The wait_until
                # hint keeps the early SP slots for the loads.
                t = tiles[i]
                xT = xt_pool.tile([128, DK, 128], BF16, tag="xT", bufs=NT)
                with tc.tile_wait_until(0.045):
                    nc.sync.dma_start_transpose(xT[:], t["xq"][:])
                t["xT"] = xT

            def emit_FC1(i):
                t = tiles[i]
                xT, st = t["xT"], t["st"]
                d1 = st[:, 8:9]
                r = big.tile([128, H], F32, tag="big")
                for half in range(2):
                    pss = [fc1_ps.tile([128, 512], F32, tag="fc1ps", name=f"fc1ps{j}") for j in range(HN // 2)]
                    for k in range(DK):
                        for j in range(HN // 2):
                            n = half * (HN // 2) + j
                            nc.tensor.matmul(
                                pss[j][:], xT[:, k, :], w1q[:, k, 512 * n:512 * (n + 1)],
                                start=(k == 0), stop=(k == DK - 1))
                    for j in range(HN // 2):
                        n = half * (HN // 2) + j
                        nc.scalar.activation(r[:, 512 * n:512 * (n + 1)], pss[j][:],
                                             AF.Relu, scale=d1)
                        if fast_act and not use_n2:
                            # per-chunk max of r (>=0) into st[:,16+n]; hmax is
                            # recovered exactly in emit_H as (max r)^2 — this
                            # moves the big hmax reduce off the H critical path
                            # into the FC1 window.
                            nc.vector.tensor_reduce(
                                st[:, 16 + n:17 + n], r[:, 512 * n:512 * (n + 1)],
                                mybir.AxisListType.X, AO.max)
                t["r"] = r

            def emit_H(i):
                # processed in column halves so the second half's stats work
                # overlaps the first half's.
                t = tiles[i]
                r, st = t["r"], t["st"]
                HH = H // 2
                halves = (slice(0, HH), slice(HH, H))
                hsum, hmax = st[:, 9:10], st[:, 10:11]
                hsum_p = [st[:, 9:10], st[:, 15:16]]
                hmax_p = [st[:, 10:11], st[:, 11:12]]
                u2, q2, rs2 = st[:, 2:3], st[:, 3:4], st[:, 4:5]
                amax2, ia2 = st[:, 5:6], st[:, 6:7]
                c2, d2 = st[:, 7:8], st[:, 1:2]
                hp = r  # in-place: hp = r*r overwrites r
                for j, sl in enumerate(halves):
                    nc.vector.tensor_tensor(hp[:, sl], r[:, sl], r[:, sl], AO.mult)
                    if not fast_act:
                        nc.vector.tensor_scalar(hp[:, sl], hp[:, sl], s_act, b_act,
                                                AO.mult, AO.add)
                    # v2 = mean(h'^2) uses h' BEFORE the norm2_w column scale
                    nc.scalar.activation(junk_ap(HH), hp[:, sl], AF.Square,
                                         accum_out=hsum_p[j])
                if use_n2:
                    hpw = big.tile([128, H], F32, tag="bign2")
                    nc.vector.tensor_tensor(hpw[:], hp[:], n2_sb[:], AO.mult)
                    hp = hpw
                if fast_act and not use_n2:
                    # hmax = (max r)^2, exact: squares are monotone on r >= 0
                    # and (max r)^2 is the same fl() expression as the max
                    # element's hp. Partial chunk maxes were computed in FC1.
                    hmaxr = st[:, 24:25]
                    nc.vector.tensor_reduce(hmaxr, st[:, 16:24],
                                            mybir.AxisListType.X, AO.max)
                    nc.vector.tensor_tensor(hmax, hmaxr, hmaxr, AO.mult)
                else:
                    for j, sl in enumerate(halves):
                        nc.vector.tensor_reduce(
                            hmax_p[j], hp[:, sl], mybir.AxisListType.X, AO.max,
                            apply_absolute_value=True if general else None)
                    nc.vector.tensor_tensor(hmax, hmax_p[0], hmax_p[1], AO.max)
                nc.vector.tensor_tensor(hsum, hsum_p[0], hsum_p[1], AO.add)
                nc.vector.tensor_scalar(u2, hsum, 1.0 / H, EPS_NORM, AO.mult, AO.add)
                rs2 = rsqrt_newton(st, u2, q2, rs2)
                nc.vector.tensor_scalar(amax2, hmax, rs2, EPS_Q, AO.mult, AO.max)
                nc.vector.reciprocal(ia2, amax2)
                nc.vector.tensor_scalar(c2, ia2, rs2, 127.0, AO.mult, AO.mult)
                nc.vector.tensor_scalar(d2, amax2, mw2_ap, 1.0 / 127.0, AO.mult, AO.mult)
                hq = hq_pool.tile([128, H], BF16, tag="hq")
                for j, sl in enumerate(halves):
                    nc.vector.tensor_scalar(hp[:, sl], hp[:, sl], c2, MAGIC,
                                            AO.mult, AO.add)
                    nc.scalar.activation(hq[:, sl], hp[:, sl], AF.Identity,
                                         bias=nmagic_b[:])
                hT = ht_pool.tile([128, HK, 128], BF16, tag="hT")
                nc.sync.dma_start_transpose(hT[:], hq[:])
                t["hT"], t["d2"] = hT, d2

            def emit_FC2a(i):
                # first k-half only: needs just w2q chunks 0..HK/2-1, so the
                # PE can start FC2 while the second half of w2 still streams.
                t = tiles[i]
                hT = t["hT"]
                t["ps2"] = []
                for n in range(DN):
                    ps2 = fc2_ps.tile([128, 512], F32, tag="fc2ps")
                    for k in range(HK // 2):
                        nc.tensor.matmul(
                            ps2[:], hT[:, k, :], w2q[:, k, 512 * n:512 * (n + 1)],
                            start=(k == 0), stop=False)
                    t["ps2"].append(ps2)

            def emit_FC2b(i):
                t = tiles[i]
                hT, d2 = t["hT"], t["d2"]
                y_sb = ymid.tile([128, D], F32, tag="y")
                for n in range(DN):
                    ps2 = t["ps2"][n]
                    for k in range(HK // 2, HK):
                        nc.tensor.matmul(
                            ps2[:], hT[:, k, :], w2q[:, k, 512 * n:512 * (n + 1)],
                            start=False, stop=(k == HK - 1))
                    nc.scalar.activation(y_sb[:, 512 * n:512 * (n + 1)], ps2[:],
                                         AF.Copy, scale=d2)
                nc.sync.dma_start(y_d[128 * i:128 * (i + 1), :], y_sb[:])

            # Emission order notes:
            #  - weight-mean partial reduce + AR-input store go first; the
            #    collective itself is emitted AFTER X(0)/X(1) so their DMA
            #    transposes precede it in program order (Tile serializes
            #    transposes against earlier collectives).
            #  - w1 piece loads are prefetched before X(0) so the data is in
            #    SBUF the moment the AllReduce lands; pieces are quantized
            #    column-half-0 first so FC1(0)'s first half can start while
            #    half-1 still streams.
            #  - FC2(i) is emitted one iteration late (at i+1) so the w2
            #    quantization and the H(i) pipeline have a full extra FC1
            #    window to complete; the PE stream stays dense:
            #    FC1(0) FC1(1) FC1(2) FC2(0) FC1(3) FC2(1) ...
            NTL = cfg.get("nt") or NT
            weight_means_reduce()
            weight_means_collective()
            w1_order = [(k, j) for j in range(2) for k in range(DK)]
            for k, j in w1_order[:cfg["ws1_bufs"]]:
                emit_w1_load(k, j)
            # All X tile loads + stats are AllReduce-independent: emit them
            # upfront so their DVE/ACT work fills the AllReduce flight window.
            for i in range(NTL):
                emit_X(i)
            weight_means_post()
            for i in range(NTL):
                emit_d1(i)
            for i in range(NTL):
                emit_XT(i)
            for idx, (k, j) in enumerate(w1_order):
                nxt = idx + cfg["ws1_bufs"]
                if nxt < len(w1_order):
                    emit_w1_load(*w1_order[nxt])
                emit_w1_quant(k, j)
            emit_FC1(0)
            for k in range(min(cfg["ws2_bufs"], HK)):
                emit_w2_load(k)

            def w2_step(k):
                if k + cfg["ws2_bufs"] < HK:
                    emit_w2_load(k + cfg["ws2_bufs"])
                emit_w2_quant(k)

            # chunks 0..15 (all FC2a needs) before the loop; the rest spread
            # across iterations 0/1 so their DVE passes don't head-block the
            # H(0)/H(1) chains in the in-order DVE queue. FC2b(0) is emitted
            # at i==1 AFTER the last chunk, so program order stays valid.
            for k in range(HK // 2):
                w2_step(k)
            for i in range(NTL):
                emit_H(i)
                if i == 0:
                    for k in range(HK // 2, 3 * HK // 4):
                        w2_step(k)
                elif i == 1:
                    for k in range(3 * HK // 4, HK):
                        w2_step(k)
                if i >= 1:
                    emit_FC2a(i - 1)
                if i + 1 < NTL:
                    emit_FC1(i + 1)
                if i >= 1:
                    emit_FC2b(i - 1)
            emit_FC2a(NTL - 1)
            emit_FC2b(NTL - 1)

    nc.compile()
    return nc


_BUILD_CACHE = {}


def _get_module(s_act, b_act, use_n1, use_n2, cfg_key=None):
    key = (s_act, b_act, use_n1, use_n2, cfg_key)
    if key not in _BUILD_CACHE:
        _BUILD_CACHE[key] = build(s_act, b_act, use_n1, use_n2)
    return _BUILD_CACHE[key]


def make_in_maps(x, norm1_w, w1, act_scale, act_bias, norm2_w, w2):
    xf = np.ascontiguousarray(np.asarray(x, np.float32).reshape(TOK_TOTAL, D))
    w1t = np.ascontiguousarray(np.asarray(w1, np.float32).T)   # [D, H]
    w2t = np.ascontiguousarray(np.asarray(w2, np.float32).T)   # [H, D]
    use_n1 = not np.all(norm1_w == 1.0)
    use_n2 = not np.all(norm2_w == 1.0)
    in_maps = []
    for c in range(N_CORES):
        m = {
            "x": xf[TOK * c:TOK * (c + 1)],
            "w1t": w1t,
            "w2t": w2t,
            "w1s": np.ascontiguousarray(w1t[(D // N_CORES) * c:(D // N_CORES) * (c + 1)]),
            "w2s": np.ascontiguousarray(w2t[(H // N_CORES) * c:(H // N_CORES) * (c + 1)]),
        }
        if use_n1:
            m["n1"] = np.asarray(norm1_w, np.float32).reshape(1, D)
        if use_n2:
            m["n2"] = np.asarray(norm2_w, np.float32).reshape(1, H)
        in_maps.append(m)
    return in_maps, use_n1, use_n2


def kernel(x, norm1_w, w1, act_scale, act_bias, norm2_w, w2):
    in_maps, use_n1, use_n2 = make_in_maps(
        x, norm1_w, w1, act_scale, act_bias, norm2_w, w2)
    s_act = float(np.asarray(act_scale).reshape(-1)[0])
    b_act = float(np.asarray(act_bias).reshape(-1)[0])
    nc = _get_module(s_act, b_act, use_n1, use_n2)
    res = run_bass_kernel_spmd(nc, in_maps, list(range(N_CORES)))
    y = np.concatenate([res.results[c]["y"] for c in range(N_CORES)], axis=0)
    return y.reshape(B, S, D).astype(np.float32)


# revision 50
# speedup vs baseline: 1.0179x; 1.0179x over previous
"""BitNet-style MLP (rmsnorm -> act-quant -> ternary fc1 -> StarReLU ->
rmsnorm -> act-quant -> ternary fc2) on 8 Trainium2 NeuronCores.

Sharding: data-parallel over tokens (8192 tokens -> 1024/core). Weights are
replicated; the global mean(|w|) statistic is computed cooperatively (each
core reduces 1/8 of each weight, then a tiny AllReduce), after which every
core quantizes the full weight set to ternary fp8 on the fly.

Numeric scheme (identical to the reference up to fp32 rounding of scales):
the fake-quantized matmuls are integer-exact, so both matmuls run on the PE
in bf16 (activations = round(x*scale), ints in [-128,127], exact in bf16)
x fp8 (ternary weights in {-1,0,1}), with exact f32 accumulation in PSUM;
per-row dequant scales are applied at PSUM eviction. Rounding uses the
(x + 1.5*2^23) - 1.5*2^23 magic-number trick (RNE, matches jnp.round).
Weight clip uses clip-BEFORE-round at +-1.49 which is exactly equivalent to
clip(round(v),-1,1) under RNE.

Engine assignment notes (perf):
 - NO elementwise work on GpSimd: GpSimd ops and DVE 2-port perf-mode ops
   take an exclusive lock on a shared SBUF port pair and fully serialize
   against each other. Everything elementwise runs on DVE or ACT (ACT has
   its own SBUF ports).
 - DMA transposes are batched: ONE dma_start_transpose per [128, N] tile
   with a 3D [128, N/128, 128] destination (instead of N/128 separate
   128x128 calls).
 - The tiny AllReduce is issued early and its (DVE) post-processing is
   emitted after the first X tiles so the in-order DVE queue doesn't stall
   on the collective.
"""

import numpy as np
from contextlib import ExitStack

import concourse.bass as bass
import concourse.tile as tile
from concourse import bacc, mybir
from concourse.bass_utils import run_bass_kernel_spmd

AO = mybir.AluOpType
AF = mybir.ActivationFunctionType
F32 = mybir.dt.float32
BF16 = mybir.dt.bfloat16
FP8 = mybir.dt.float8e4

MAGIC = 1.5 * 2 ** 23  # RNE integer rounding for |v| <= 2^22
CLIP = 1.49            # clip-before-round bound; == clip(round(v),-1,1) after RNE
N_CORES = 8
B, S, D, H = 4, 2048, 1024, 4096
TOK_TOTAL = B * S                 # 8192
TOK = TOK_TOTAL // N_CORES        # 1024 tokens per core
NT = TOK // 128                   # 8 token tiles per core
DK = D // 128                     # 8 k-tiles for fc1
HK = H // 128                     # 32 k-tiles for fc2
HN = H // 512                     # 8 n-chunks for fc1
DN = D // 512                     # 2 n-chunks for fc2
EPS_NORM = 1e-08
EPS_Q = 1e-05

DEFAULT_CFG = dict(
    big_bufs=2,
    xmid_bufs=2,
    y_bufs=2,
    stats_bufs=6,
    ws1_bufs=3,   # shared "wst" tag ring: slice tiles + w1 half-row pieces
    ws2_bufs=2,
    hq_bufs=2,
    # 8 xq bufs: X(2..7) transposes serialize against the collective (Tile
    # guards DMA-transpose vs collective), so xq tiles pile up until the
    # AllReduce lands; the X ACT/DVE pipeline must not block on the ring.
    xq_bufs=6,
    xt_bufs=3,
    ht_bufs=2,
    fc1_ps_bufs=5,
    fc2_ps_bufs=2,
)


def _step0_free_ap(t_ap: bass.AP, count: int) -> bass.AP:
    """AP that 'writes' `count` elements per partition all landing on the
    same address (free step 0) - a bit bucket for ops whose only useful
    output is accum_out."""
    part = t_ap.ap[0]
    return bass.AP(t_ap.tensor, t_ap.offset, [[part[0], part[1]], [0, count]])


def build(s_act: float, b_act: float, use_n1: bool, use_n2: bool, cfg=None):
    """Build the per-core Bass module (SPMD: same module on all 8 cores)."""
    cfg = dict(DEFAULT_CFG, **(cfg or {}))
    general = use_n1 or use_n2 or (b_act != 0.0) or (s_act < 0.0)
    fast_act = (b_act == 0.0) and (s_act >= 0.0)

    nc = bacc.Bacc(None, target_bir_lowering=False)

    x_d = nc.dram_tensor("x", [TOK, D], F32, kind="ExternalInput")
    w1t_d = nc.dram_tensor("w1t", [D, H], F32, kind="ExternalInput")
    w2t_d = nc.dram_tensor("w2t", [H, D], F32, kind="ExternalInput")
    w1s_d = nc.dram_tensor("w1s", [D // N_CORES, H], F32, kind="ExternalInput")
    w2s_d = nc.dram_tensor("w2s", [H // N_CORES, D], F32, kind="ExternalInput")
    if use_n1:
        n1_d = nc.dram_tensor("n1", [1, D], F32, kind="ExternalInput")
    if use_n2:
        n2_d = nc.dram_tensor("n2", [1, H], F32, kind="ExternalInput")
    y_d = nc.dram_tensor("y", [TOK, D], F32, kind="ExternalOutput")

    with tile.TileContext(nc) as tc:
        with ExitStack() as ctx:
            const = ctx.enter_context(tc.tile_pool(name="const", bufs=1))
            stats = ctx.enter_context(tc.tile_pool(name="stats", bufs=cfg["stats_bufs"]))
            wq_pool = ctx.enter_context(tc.tile_pool(name="wq", bufs=1))
            ws1 = ctx.enter_context(tc.tile_pool(name="ws1", bufs=cfg["ws1_bufs"]))
            ws2 = ctx.enter_context(tc.tile_pool(name="ws2", bufs=cfg["ws2_bufs"]))
            big = ctx.enter_context(tc.tile_pool(name="big", bufs=cfg["big_bufs"]))
            xmid = ctx.enter_context(tc.tile_pool(name="xmid", bufs=cfg["xmid_bufs"]))
            ymid = ctx.enter_context(tc.tile_pool(name="ymid", bufs=cfg["y_bufs"]))
            xq_pool = ctx.enter_context(tc.tile_pool(name="xq", bufs=cfg["xq_bufs"]))
            xt_pool = ctx.enter_context(tc.tile_pool(name="xT", bufs=cfg["xt_bufs"]))
            hq_pool = ctx.enter_context(tc.tile_pool(name="hq", bufs=cfg["hq_bufs"]))
            ht_pool = ctx.enter_context(tc.tile_pool(name="hT", bufs=cfg["ht_bufs"]))
            junk_pool = ctx.enter_context(tc.tile_pool(name="junk", bufs=1))
            fc1_ps = ctx.enter_context(tc.tile_pool(name="fc1ps", bufs=cfg["fc1_ps_bufs"], space="PSUM"))
            fc2_ps = ctx.enter_context(tc.tile_pool(name="fc2ps", bufs=cfg["fc2_ps_bufs"], space="PSUM"))
            ps_small = ctx.enter_context(tc.tile_pool(name="pssmall", bufs=1, space="PSUM"))
            dram = ctx.enter_context(tc.tile_pool(name="dram", bufs=1, space="DRAM"))

            ones_col = const.tile([128, 1], F32)
            nc.vector.memset(ones_col[:], 1.0)
            ones_row = const.tile([1, 128], F32)
            nc.vector.memset(ones_row[:], 1.0)
            magic_b = const.tile([128, 1], F32)
            nc.vector.memset(magic_b[:], MAGIC)
            nmagic_b = const.tile([128, 1], F32)
            nc.vector.memset(nmagic_b[:], -MAGIC)

            # wstats columns: 0=mw1, 1=mw2, 2=s1, 3=s2
            wstats = stats.tile([128, 4], F32, tag="wstats")
            mw1_ap, mw2_ap = wstats[:, 0:1], wstats[:, 1:2]
            s1_ap, s2_ap = wstats[:, 2:3], wstats[:, 3:4]

            junk_small = junk_pool.tile([128, 8], BF16, tag="junk")

            def junk_ap(n):
                return _step0_free_ap(junk_small[:, 0:1], n)

            if use_n1:
                n1_sb = const.tile([128, D], F32)
                a0 = n1_d[:]
                nc.sync.dma_start(n1_sb[:], bass.AP(a0.tensor, a0.offset, [[0, 128], [1, D]]))
            if use_n2:
                n2_sb = const.tile([128, H], F32)
                a0 = n2_d[:]
                nc.sync.dma_start(n2_sb[:], bass.AP(a0.tensor, a0.offset, [[0, 128], [1, H]]))

            # -------- W phase: global mean(|w|) via ONE tiny AllReduce --------
            # In steady state the collective doubles as a barrier that
            # re-aligns the 8 cores each iteration, so its cost is the true
            # AR latency (~20-40us), not the first-exec launch skew.
            ar_in = dram.tile([4, 1], F32, tag="ari")
            ar_out = dram.tile([4, 1], F32, tag="aro")

            def weight_means_reduce():
                # 4 pipelined load+reduce pieces (w1 halves -> cols 0/2,
                # w2 halves -> cols 1/3) so the AllReduce input is ready
                # ~15us in instead of ~30us.
                # 4 pipelined load+reduce pieces (w1 halves -> cols 0/2,
                # w2 halves -> cols 1/3) so the AllReduce input is ready
                # ~15us in instead of ~30us.
                parts = stats.tile([128, 4], F32, tag="wpart")
                pieces = []
                halfc = w1s_d.shape[1] // 2      # w1s split by columns (128 rows)
                half2 = w2s_d.shape[0] // 2      # w2s split by rows (512 rows)
                pieces.append((0, w1s_d[:, 0:halfc]))
                pieces.append((1, w2s_d[0:half2, :]))
                pieces.append((2, w1s_d[:, halfc:]))
                pieces.append((3, w2s_d[half2:, :]))
                for col, sl in pieces:
                    st_t = ws1.tile([128, sl.shape[0] // 128, sl.shape[1]],
                                    F32, tag="wst", name=f"wst{col}")
                    nc.sync.dma_start(st_t[:], sl.rearrange("(c p) f -> p c f", p=128))
                    nc.vector.tensor_reduce(parts[:, col:col + 1], st_t[:],
                                            mybir.AxisListType.XY, AO.add,
                                            apply_absolute_value=True)
                # cross-partition sums: [4,1] = parts.T @ ones
                pss = ps_small.tile([128, 1], F32, tag="ps", name="pssum")
                nc.tensor.matmul(pss[0:4, :], parts[:], ones_col[:], start=True, stop=True)
                sb1 = stats.tile([4, 1], F32, tag="wsum")
                nc.scalar.activation(sb1[:], pss[0:4, :], AF.Copy)
                nc.sync.dma_start(ar_in[:], sb1[:])

            def weight_means_collective():
                nc.gpsimd.collective_compute(
                    "AllReduce", AO.add,
                    replica_groups=[list(range(N_CORES))],
                    ins=[ar_in[:].opt()], outs=[ar_out[:].opt()],
                )

            def weight_means_post():
                bc = stats.tile([128, 4], F32, tag="wbc")
                a0 = ar_out[:]
                # issued on the ACT HWDGE ring: the SP ring is jammed with
                # DMA-transposes that serialize against the collective.
                nc.scalar.dma_start(bc[:], bass.AP(a0.tensor, a0.offset, [[0, 128], [1, 4]]))
                bsum = stats.tile([128, 2], F32, tag="wbsum")
                nc.vector.tensor_tensor(bsum[:], bc[:, 0:2], bc[:, 2:4], AO.add)
                nc.vector.tensor_scalar(wstats[:, 0:2], bsum[:], 1.0 / (D * H), EPS_Q,
                                        AO.mult, AO.max)
                nc.vector.reciprocal(wstats[:, 2:4], wstats[:, 0:2])

            # ---------------- weight quantization (full, streamed) -----------
            # 3 passes, zero GpSimd:
            #   DVE: v = min(w*s, 1.49)        (2-op tensor_scalar, in-place)
            #   DVE: v = max(v, -1.49) + MAGIC (2-op tensor_scalar, in-place)
            #   ACT: wq = v - MAGIC -> fp8     (Identity with bias=-MAGIC)
            # w1 is processed in [128, H/2] half-row pieces so FC1(0)'s k-loop
            # can start consuming quantized chunks while later ones stream.
            # Loads are decoupled from quants so DMAs prefetch `bufs` deep.
            w1q = wq_pool.tile([128, DK, H], FP8, tag="w1q")
            w2q = wq_pool.tile([128, HK, D], FP8, tag="w2q")
            HH2 = H // 2
            w1_tiles = {}
            w2_tiles = {}

            def emit_w1_load(k, j):
                wt = ws1.tile([128, HH2], F32, tag="wst")
                nc.sync.dma_start(wt[:], w1t_d[128 * k:128 * (k + 1),
                                               HH2 * j:HH2 * (j + 1)])
                w1_tiles[(k, j)] = wt

            def emit_w1_quant(k, j):
                # quantize in 512-col quarters (the exact region one FC1
                # matmul reads): each matmul unblocks on its own quarter,
                # keeping PE stalls under the 3.4us HAM re-throttle window
                # while the weight stream paces the pipeline.
                wt = w1_tiles.pop((k, j))
                for q in range(HH2 // 512):
                    sl = slice(512 * q, 512 * (q + 1))
                    nc.vector.tensor_scalar(wt[:, sl], wt[:, sl], s1_ap, CLIP,
                                            AO.mult, AO.min)
                    nc.vector.tensor_scalar(wt[:, sl], wt[:, sl], -CLIP, MAGIC,
                                            AO.max, AO.add)
                    nc.scalar.activation(
                        w1q[:, k, HH2 * j + 512 * q:HH2 * j + 512 * (q + 1)],
                        wt[:, sl], AF.Identity, bias=nmagic_b[:])

            def emit_w2_load(k):
                wt = ws2.tile([128, D], F32, tag="ws2")
                nc.sync.dma_start(wt[:], w2t_d[128 * k:128 * (k + 1), :])
                w2_tiles[k] = wt

            def emit_w2_quant(k):
                # 512-col halves for the same reason as emit_w1_quant.
                wt = w2_tiles.pop(k)
                for q in range(D // 512):
                    sl = slice(512 * q, 512 * (q + 1))
                    nc.vector.tensor_scalar(wt[:, sl], wt[:, sl], s2_ap, CLIP,
                                            AO.mult, AO.min)
                    nc.vector.tensor_scalar(wt[:, sl], wt[:, sl], -CLIP, MAGIC,
                                            AO.max, AO.add)
                    nc.scalar.activation(w2q[:, k, sl], wt[:, sl],
                                         AF.Identity, bias=nmagic_b[:])

            # ---------------- main token-tile pipeline ------------------------
            tiles = [dict() for _ in range(NT)]

            def rsqrt_newton(st, u, q, rs0):
                """rs = 1/sqrt(u), Newton-refined to ~1ulp (ACT Sqrt alone is
                only ~8ulp and rounding-boundary flips are costly here)."""
                nA, nB, rsr = st[:, 12:13], st[:, 13:14], st[:, 14:15]
                nc.scalar.activation(q, u, AF.Sqrt)
                nc.vector.reciprocal(rs0, q)
                nc.vector.tensor_tensor(nA, rs0, rs0, AO.mult)
                nc.vector.tensor_tensor(nB, nA, u, AO.mult)
                nc.vector.tensor_scalar(nA, nB, -0.5, 1.5, AO.mult, AO.add)
                nc.vector.tensor_tensor(rsr, rs0, nA, AO.mult)
                return rsr

            def emit_d1(i):
                # d1 = amax * mw1 * k; depends on the AllReduce (mw1), so for
                # tiles 0/1 it is emitted from weight_means_post, for later
                # tiles at the end of emit_X.
                t = tiles[i]
                st = t["st"]
                amax, d1 = st[:, 5:6], st[:, 8:9]
                d1_k = (np.sqrt(s_act) if fast_act else 1.0) / 127.0
                nc.vector.tensor_scalar(d1, amax, mw1_ap, d1_k, AO.mult, AO.mult)

            def emit_X(i):
                t = tiles[i]
                xt = xmid.tile([128, D], F32, tag="x")
                nc.sync.dma_start(xt[:], x_d[128 * i:128 * (i + 1), :])
                st = stats.tile([128, 32], F32, tag="st", bufs=NT)
                ssum, m1 = st[:, 0:1], st[:, 1:2]
                u, q, rs = st[:, 2:3], st[:, 3:4], st[:, 4:5]
                amax, ia = st[:, 5:6], st[:, 6:7]
                c1, d1 = st[:, 7:8], st[:, 8:9]
                nc.scalar.activation(junk_ap(D), xt[:], AF.Square, accum_out=ssum)
                if use_n1:
                    xw = xmid.tile([128, D], F32, tag="xw")
                    nc.vector.tensor_tensor(xw[:], xt[:], n1_sb[:], AO.mult)
                    quant_src = xw
                else:
                    quant_src = xt
                nc.vector.tensor_reduce(m1, quant_src[:], mybir.AxisListType.X,
                                        AO.max, apply_absolute_value=True)
                nc.vector.tensor_scalar(u, ssum, 1.0 / D, EPS_NORM, AO.mult, AO.add)
                rs = rsqrt_newton(st, u, q, rs)
                nc.vector.tensor_scalar(amax, m1, rs, EPS_Q, AO.mult, AO.max)
                nc.vector.reciprocal(ia, amax)
                nc.vector.tensor_scalar(c1, ia, rs, 127.0, AO.mult, AO.mult)
                nc.vector.tensor_scalar(quant_src[:], quant_src[:], c1, MAGIC, AO.mult, AO.add)
                xq = xq_pool.tile([128, D], BF16, tag="xq")
                nc.scalar.activation(xq[:], quant_src[:], AF.Identity, bias=nmagic_b[:])
                t["xq"], t["st"], t["d1"] = xq, st, d1

            def emit_XT(i):
                # transposes serialize against the collective (Tile guard):
                # without a timing hint the scheduler slots them EARLY on the
                # SP ring (its cost model assumes the collective is instant),
                # blocking the x loads behind the AllReduce. The wait_until
                # hint keeps the early SP slots for the loads.
                t = tiles[i]
                xT = xt_pool.tile([128, DK, 128], BF16, tag="xT", bufs=NT)
                with tc.tile_wait_until(0.045):
                    nc.sync.dma_start_transpose(xT[:], t["xq"][:])
                t["xT"] = xT

            def emit_FC1(i):
                t = tiles[i]
                xT, st = t["xT"], t["st"]
                d1 = st[:, 8:9]
                r = big.tile([128, H], F32, tag="big")
                for half in range(2):
                    pss = [fc1_ps.tile([128, 512], F32, tag="fc1ps", name=f"fc1ps{j}") for j in range(HN // 2)]
                    for k in range(DK):
                        for j in range(HN // 2):
                            n = half * (HN // 2) + j
                            nc.tensor.matmul(
                                pss[j][:], xT[:, k, :], w1q[:, k, 512 * n:512 * (n + 1)],
                                start=(k == 0), stop=(k == DK - 1))
                    for j in range(HN // 2):
                        n = half * (HN // 2) + j
                        nc.scalar.activation(r[:, 512 * n:512 * (n + 1)], pss[j][:],
                                             AF.Relu, scale=d1)
                        if fast_act and not use_n2:
                            # per-chunk max of r (>=0) into st[:,16+n]; hmax is
                            # recovered exactly in emit_H as (max r)^2 — this
                            # moves the big hmax reduce off the H critical path
                            # into the FC1 window.
                            nc.vector.tensor_reduce(
                                st[:, 16 + n:17 + n], r[:, 512 * n:512 * (n + 1)],
                                mybir.AxisListType.X, AO.max)
                t["r"] = r

            def emit_H(i):
                # processed in column halves so the second half's stats work
                # overlaps the first half's.
                t = tiles[i]
                r, st = t["r"], t["st"]
                HH = H // 2
                halves = (slice(0, HH), slice(HH, H))
                hsum, hmax = st[:, 9:10], st[:, 10:11]
                hsum_p = [st[:, 9:10], st[:, 15:16]]
                hmax_p = [st[:, 10:11], st[:, 11:12]]
                u2, q2, rs2 = st[:, 2:3], st[:, 3:4], st[:, 4:5]
                amax2, ia2 = st[:, 5:6], st[:, 6:7]
                c2, d2 = st[:, 7:8], st[:, 1:2]
                hp = r  # in-place: hp = r*r overwrites r
                for j, sl in enumerate(halves):
                    nc.vector.tensor_tensor(hp[:, sl], r[:, sl], r[:, sl], AO.mult)
                    if not fast_act:
                        nc.vector.tensor_scalar(hp[:, sl], hp[:, sl], s_act, b_act,
                                                AO.mult, AO.add)
                    # v2 = mean(h'^2) uses h' BEFORE the norm2_w column scale
                    nc.scalar.activation(junk_ap(HH), hp[:, sl], AF.Square,
                                         accum_out=hsum_p[j])
                if use_n2:
                    hpw = big.tile([128, H], F32, tag="bign2")
                    nc.vector.tensor_tensor(hpw[:], hp[:], n2_sb[:], AO.mult)
                    hp = hpw
                if fast_act and not use_n2:
                    # hmax = (max r)^2, exact: squares are monotone on r >= 0
                    # and (max r)^2 is the same fl() expression as the max
                    # element's hp. Partial chunk maxes were computed in FC1.
                    hmaxr = st[:, 24:25]
                    nc.vector.tensor_reduce(hmaxr, st[:, 16:24],
                                            mybir.AxisListType.X, AO.max)
                    nc.vector.tensor_tensor(hmax, hmaxr, hmaxr, AO.mult)
                else:
                    for j, sl in enumerate(halves):
                        nc.vector.tensor_reduce(
                            hmax_p[j], hp[:, sl], mybir.AxisListType.X, AO.max,
                            apply_absolute_value=True if general else None)
                    nc.vector.tensor_tensor(hmax, hmax_p[0], hmax_p[1], AO.max)
                nc.vector.tensor_tensor(hsum, hsum_p[0], hsum_p[1], AO.add)
                nc.vector.tensor_scalar(u2, hsum, 1.0 / H, EPS_NORM, AO.mult, AO.add)
                rs2 = rsqrt_newton(st, u2, q2, rs2)
                nc.vector.tensor_scalar(amax2, hmax, rs2, EPS_Q, AO.mult, AO.max)
                nc.vector.reciprocal(ia2, amax2)
                nc.vector.tensor_scalar(c2, ia2, rs2, 127.0, AO.mult, AO.mult)
                nc.vector.tensor_scalar(d2, amax2, mw2_ap, 1.0 / 127.0, AO.mult, AO.mult)
                hq = hq_pool.tile([128, H], BF16, tag="hq")
                for j, sl in enumerate(halves):
                    nc.vector.tensor_scalar(hp[:, sl], hp[:, sl], c2, MAGIC,
                                            AO.mult, AO.add)
                    nc.scalar.activation(hq[:, sl], hp[:, sl], AF.Identity,
                                         bias=nmagic_b[:])
                hT = ht_pool.tile([128, HK, 128], BF16, tag="hT")
                nc.sync.dma_start_transpose(hT[:], hq[:])
                t["hT"], t["d2"] = hT, d2

            def emit_FC2a(i):
                # first k-half only: needs just w2q chunks 0..HK/2-1, so the
                # PE can start FC2 while the second half of w2 still streams.
                t = tiles[i]
                hT = t["hT"]
                t["ps2"] = []
                for n in range(DN):
                    ps2 = fc2_ps.tile([128, 512], F32, tag="fc2ps")
                    for k in range(HK // 2):
                        nc.tensor.matmul(
                            ps2[:], hT[:, k, :], w2q[:, k, 512 * n:512 * (n + 1)],
                            start=(k == 0), stop=False)
                    t["ps2"].append(ps2)

            def emit_FC2b(i):
                t = tiles[i]
                hT, d2 = t["hT"], t["d2"]
                y_sb = ymid.tile([128, D], F32, tag="y")
                for n in range(DN):
                    ps2 = t["ps2"][n]
                    for k in range(HK // 2, HK):
                        nc.tensor.matmul(
                            ps2[:], hT[:, k, :], w2q[:, k, 512 * n:512 * (n + 1)],
                            start=False, stop=(k == HK - 1))
                    nc.scalar.activation(y_sb[:, 512 * n:512 * (n + 1)], ps2[:],
                                         AF.Copy, scale=d2)
                nc.sync.dma_start(y_d[128 * i:128 * (i + 1), :], y_sb[:])

            # Emission order notes:
            #  - weight-mean partial reduce + AR-input store go first; the
            #    collective itself is emitted AFTER X(0)/X(1) so their DMA
            #    transposes precede it in program order (Tile serializes
            #    transposes against earlier collectives).
            #  - w1 piece loads are prefetched before X(0) so the data is in
            #    SBUF the moment the AllReduce lands; pieces are quantized
            #    column-half-0 first so FC1(0)'s first half can start while
            #    half-1 still streams.
            #  - FC2(i) is emitted one iteration late (at i+1) so the w2
            #    quantization and the H(i) pipeline have a full extra FC1
            #    window to complete; the PE stream stays dense:
            #    FC1(0) FC1(1) FC1(2) FC2(0) FC1(3) FC2(1) ...
            NTL = cfg.get("nt") or NT
            weight_means_reduce()
            weight_means_collective()
            w1_order = [(k, j) for j in range(2) for k in range(DK)]
            for k, j in w1_order[:cfg["ws1_bufs"]]:
                emit_w1_load(k, j)
            # All X tile loads + stats are AllReduce-independent: emit them
            # upfront so their DVE/ACT work fills the AllReduce flight window.
            for i in range(NTL):
                emit_X(i)
            weight_means_post()
            for i in range(NTL):
                emit_d1(i)
            for i in range(NTL):
                emit_XT(i)
            for idx, (k, j) in enumerate(w1_order):
                nxt = idx + cfg["ws1_bufs"]
                if nxt < len(w1_order):
                    emit_w1_load(*w1_order[nxt])
                emit_w1_quant(k, j)
            emit_FC1(0)
            for k in range(min(cfg["ws2_bufs"], HK)):
                emit_w2_load(k)

            def w2_step(k):
                if k + cfg["ws2_bufs"] < HK:
                    emit_w2_load(k + cfg["ws2_bufs"])
                emit_w2_quant(k)

            # chunks 0..15 (all FC2a needs) before the loop; the rest spread
            # across iterations 0/1 so their DVE passes don't head-block the
            # H(0)/H(1) chains in the in-order DVE queue. FC2b(0) is emitted
            # at i==1 AFTER the last chunk, so program order stays valid.
            for k in range(HK // 2):
                w2_step(k)
            for i in range(NTL):
                emit_H(i)
                if i == 0:
                    for k in range(HK // 2, 3 * HK // 4):
                        w2_step(k)
                elif i == 1:
                    for k in range(3 * HK // 4, HK):
                        w2_step(k)
                if i >= 1:
                    emit_FC2a(i - 1)
                if i + 1 < NTL:
                    emit_FC1(i + 1)
                if i >= 1:
                    emit_FC2b(i - 1)
            emit_FC2a(NTL - 1)
            emit_FC2b(NTL - 1)

    nc.compile()
    return nc


_BUILD_CACHE = {}


def _get_module(s_act, b_act, use_n1, use_n2, cfg_key=None):
    key = (s_act, b_act, use_n1, use_n2, cfg_key)
    if key not in _BUILD_CACHE:
        _BUILD_CACHE[key] = build(s_act, b_act, use_n1, use_n2)
    return _BUILD_CACHE[key]


def make_in_maps(x, norm1_w, w1, act_scale, act_bias, norm2_w, w2):
    xf = np.ascontiguousarray(np.asarray(x, np.float32).reshape(TOK_TOTAL, D))
    w1t = np.ascontiguousarray(np.asarray(w1, np.float32).T)   # [D, H]
    w2t = np.ascontiguousarray(np.asarray(w2, np.float32).T)   # [H, D]
    use_n1 = not np.all(norm1_w == 1.0)
    use_n2 = not np.all(norm2_w == 1.0)
    in_maps = []
    for c in range(N_CORES):
        m = {
            "x": xf[TOK * c:TOK * (c + 1)],
            "w1t": w1t,
            "w2t": w2t,
            "w1s": np.ascontiguousarray(w1t[(D // N_CORES) * c:(D // N_CORES) * (c + 1)]),
            "w2s": np.ascontiguousarray(w2t[(H // N_CORES) * c:(H // N_CORES) * (c + 1)]),
        }
        if use_n1:
            m["n1"] = np.asarray(norm1_w, np.float32).reshape(1, D)
        if use_n2:
            m["n2"] = np.asarray(norm2_w, np.float32).reshape(1, H)
        in_maps.append(m)
    return in_maps, use_n1, use_n2


def kernel(x, norm1_w, w1, act_scale, act_bias, norm2_w, w2):
    in_maps, use_n1, use_n2 = make_in_maps(
        x, norm1_w, w1, act_scale, act_bias, norm2_w, w2)
    s_act = float(np.asarray(act_scale).reshape(-1)[0])
    b_act = float(np.asarray(act_bias).reshape(-1)[0])
    nc = _get_module(s_act, b_act, use_n1, use_n2)
    res = run_bass_kernel_spmd(nc, in_maps, list(range(N_CORES)))
    y = np.concatenate([res.results[c]["y"] for c in range(N_CORES)], axis=0)
    return y.reshape(B, S, D).astype(np.float32)
